# revision 6
# baseline (speedup 1.0000x reference)
"""MLA (multi-head latent attention) prefill block on 8 Trainium2 NeuronCores.

Tensor-parallel over heads: each core computes 4 of the 32 heads end-to-end.
The kv latent path (kv_a projection, rms-norm, rope) is replicated on every
core. Per-core partial outputs (row-parallel wo matmul) are summed on host.

v2: non-absorbed prefill attention. Instead of absorbing wkv_b into q
(contraction 512+64 per score), materialize per-head k_nopeT = wbk^T kvcT
[128, S] and v = kvcT^T wbv [S, 128] once (cheap: KVL*DN*S per head), then
scores contract over 128+64 and the attention output over t directly into
DV=128 (not KVL=512). This cuts attention-phase PE work ~2.6x.

All matmul operands are bf16 (PSUM accumulation stays fp32); rel-err budget
is 2e-2 so bf16's ~0.4% is safe. Everything is computed in transposed
[feature, seq] layouts; no PE transposes and no DRAM scratch are needed.

Self-contained: hardcodes all shapes from the problem spec.
"""

from contextlib import ExitStack

import numpy as np
import ml_dtypes

import concourse.bacc as bacc
import concourse.bass as bass  # noqa: F401
import concourse.mybir as mybir
import concourse.tile as tile
from concourse.bass_utils import run_bass_kernel_spmd

# ---- problem constants ----
DIM = 2048
NH = 32
DN = 128   # qk_nope_head_dim
DR = 64    # qk_rope_head_dim
DV = 128   # v_head_dim
KVL = 512  # kv_lora_rank
S = 2048   # sequence length (B=1)
SCALE = float((DN + DR) ** -0.5)
EPS = 1e-6

NCORES = 8
NHC = NH // NCORES      # heads per core = 4
P = 128                 # partitions
SF = 512                # free-dim tile (s tiles)
NST = S // SF           # 4 s tiles
NTT = S // P            # 16 t tiles
NDC = DIM // P          # 16 contraction chunks over model dim
NCC = KVL // P          # 4 latent chunks

F32 = mybir.dt.float32
BF16 = mybir.dt.bfloat16
NPBF16 = ml_dtypes.bfloat16


def build_nc(repeat=1):
    """Build the per-core Bass program (identical on all 8 cores)."""
    nc = bacc.Bacc("TRN2", target_bir_lowering=False, debug=False,
                   num_devices=NCORES)

    # ---- DRAM I/O ----
    d_xT = nc.dram_tensor("xT", [DIM, S], BF16, kind="ExternalInput")
    d_wqn = nc.dram_tensor("wq_n", [DIM, NHC * DN], BF16,
                           kind="ExternalInput")
    d_wqpr = nc.dram_tensor("wq_pr", [DIM, NHC * 32], BF16,
                            kind="ExternalInput")
    d_wqpi = nc.dram_tensor("wq_pi", [DIM, NHC * 32], BF16,
                            kind="ExternalInput")
    # reordered: [512 latent | 32 rope-even | 32 rope-odd]
    d_wkva = nc.dram_tensor("wkv_ar", [DIM, KVL + DR], BF16,
                            kind="ExternalInput")
    d_wbkT = nc.dram_tensor("wbkT", [NHC, KVL, DN], BF16,
                            kind="ExternalInput")
    d_wbvT = nc.dram_tensor("wbvT", [NHC, KVL, DV], BF16,
                            kind="ExternalInput")
    d_wo = nc.dram_tensor("wo_c", [NHC * DV, DIM], BF16,
                          kind="ExternalInput")
    d_cosr = nc.dram_tensor("cosR", [P, S], F32, kind="ExternalInput")
    d_sinr = nc.dram_tensor("sinR", [P, S], F32, kind="ExternalInput")
    d_out = nc.dram_tensor("outT", [DIM, S], BF16,
                            kind="ExternalOutput")

    out = d_out.ap()

    with tile.TileContext(nc) as tc:
      for _rep in range(repeat):
        with ExitStack() as top:
            cst = top.enter_context(tc.tile_pool(name="const", bufs=1))
            ones_c = cst.tile([P, 1], BF16, tag="ones_c", name="ones_c")
            nc.gpsimd.memset(ones_c[:], 1.0)
            epsb = cst.tile([1, 1], F32, tag="epsb", name="epsb")
            nc.gpsimd.memset(epsb[:], EPS)

            # long-lived activations (bf16, [feature, seq] layouts)
            kvp = top.enter_context(tc.tile_pool(name="kvT", bufs=NCC))
            kvcT = [kvp.tile([P, S], BF16, tag="kvcT", name="kvcT")
                    for _ in range(NCC)]
            kpp = top.enter_context(tc.tile_pool(name="kpT", bufs=1))
            kpeT = kpp.tile([DR, S], BF16, tag="kpeT", name="kpeT")
            qnp = top.enter_context(tc.tile_pool(name="qn", bufs=NHC))
            qn = [qnp.tile([DN, S], BF16, tag="qn", name="qn")
                  for _ in range(NHC)]
            qpp = top.enter_context(tc.tile_pool(name="qp", bufs=NHC))
            qp = [qpp.tile([DR, S], BF16, tag="qp", name="qp")
                  for _ in range(NHC)]
            otp = top.enter_context(tc.tile_pool(name="oT", bufs=NHC))
            oTs = [otp.tile([DV, S], BF16, tag="oT", name="oT")
                   for _ in range(NHC)]

            # ===== phase 1: q + kv projections, single pass over xT ========
            with ExitStack() as ph1:
                wrp = ph1.enter_context(tc.tile_pool(name="wres", bufs=1))
                xsl = ph1.enter_context(tc.tile_pool(name="xsl", bufs=8))
                wqn_a = wrp.tile([P, NDC * NHC * DN], BF16, tag="wqn",
                                 name="wqn")
                wqpr_a = wrp.tile([P, NDC * NHC * 32], BF16, tag="wqpr",
                                  name="wqpr")
                wqpi_a = wrp.tile([P, NDC * NHC * 32], BF16, tag="wqpi",
                                  name="wqpi")
                wkva_a = wrp.tile([P, NDC * (KVL + DR)], BF16, tag="wkva",
                                  name="wkva")
                cosR = wrp.tile([P, S], F32, tag="cosR", name="cosR")
                sinR = wrp.tile([P, S], F32, tag="sinR", name="sinR")
                nc.sync.dma_start(cosR[:], d_cosr.ap())
                nc.sync.dma_start(sinR[:], d_sinr.ap())

                xh0 = [xsl.tile([P, 4 * SF], BF16, tag="xsl", name="xsl")
                       for _ in range(4)]
                xTj0 = d_xT.ap()[:, 0:SF].rearrange("(d p) f -> p d f", p=P)
                # interleave x and weight quarters in consumption order
                for q4 in range(4):
                    hd = slice(q4 * (NDC // 4), (q4 + 1) * (NDC // 4))
                    nc.sync.dma_start(
                        xh0[q4][:].rearrange("p (d f) -> p d f", d=4),
                        xTj0[:, 4 * q4:4 * (q4 + 1)])
                    nc.sync.dma_start(
                        wqn_a[:].rearrange("p (d c) -> p d c", d=NDC)[:, hd],
                        d_wqn.ap().rearrange("(d p) c -> p d c", p=P)[:, hd])
                    nc.sync.dma_start(
                        wqpr_a[:].rearrange("p (d c) -> p d c", d=NDC)[:, hd],
                        d_wqpr.ap().rearrange("(d p) c -> p d c", p=P)[:, hd])
                    nc.sync.dma_start(
                        wqpi_a[:].rearrange("p (d c) -> p d c", d=NDC)[:, hd],
                        d_wqpi.ap().rearrange("(d p) c -> p d c", p=P)[:, hd])
                    nc.sync.dma_start(
                        wkva_a[:].rearrange("p (d c) -> p d c", d=NDC)[:, hd],
                        d_wkva.ap().rearrange("(d p) c -> p d c", p=P)[:, hd])

                rts = ph1.enter_context(tc.tile_pool(name="ropetmp", bufs=2))
                rox = ph1.enter_context(tc.tile_pool(name="ropeout", bufs=2))
                sqs = ph1.enter_context(tc.tile_pool(name="sqs", bufs=2))
                nrm = ph1.enter_context(tc.tile_pool(name="nrm", bufs=2))
                qac = ph1.enter_context(
                    tc.tile_pool(name="acc1", bufs=8, space="PSUM"))

                for j in range(NST):
                    js = slice(j * SF, (j + 1) * SF)
                    if j == 0:
                        xh = xh0
                    else:
                        xTj = d_xT.ap()[:, js].rearrange(
                            "(d p) f -> p d f", p=P)
                        xh = [xsl.tile([P, 4 * SF], BF16, tag="xsl",
                                       name="xsl") for _ in range(4)]
                        for q4 in range(4):
                            nc.sync.dma_start(
                                xh[q4][:].rearrange("p (d f) -> p d f", d=4),
                                xTj[:, 4 * q4:4 * (q4 + 1)])

                    # ---- pass A: q projections for this s block ----
                    pss = [qac.tile([P, SF], F32, tag="acc", name="acc")
                           for _ in range(NHC + 2)]
                    for d in range(NDC):
                        xs = xh[d // 4][:, (d % 4) * SF:(d % 4 + 1) * SF]
                        for h in range(NHC):
                            nc.tensor.matmul(
                                pss[h][:],
                                wqn_a[:, d * NHC * DN + h * DN:
                                      d * NHC * DN + (h + 1) * DN],
                                xs, start=(d == 0), stop=(d == NDC - 1))
                        nc.tensor.matmul(
                            pss[NHC][:], wqpr_a[:, d * P:(d + 1) * P], xs,
                            start=(d == 0), stop=(d == NDC - 1))
                        nc.tensor.matmul(
                            pss[NHC + 1][:], wqpi_a[:, d * P:(d + 1) * P],
                            xs, start=(d == 0), stop=(d == NDC - 1))
                    for h in range(NHC):
                        nc.scalar.copy(qn[h][:, js], pss[h][:])
                    # rope rotation for q_pe (tiles hold r of 4 heads | i)
                    t1 = rts.tile([P, SF], F32, tag="t1", name="t1")
                    t2 = rts.tile([P, SF], F32, tag="t2", name="t2")
                    ror = rox.tile([P, SF], BF16, tag="ror", name="ror")
                    roi = rox.tile([P, SF], BF16, tag="roi", name="roi")
                    nc.vector.tensor_mul(t1[:], pss[NHC][:], cosR[:, js])
                    nc.vector.tensor_mul(t2[:], pss[NHC + 1][:], sinR[:, js])
                    nc.vector.tensor_sub(ror[:], t1[:], t2[:])
                    nc.vector.tensor_mul(t1[:], pss[NHC][:], sinR[:, js])
                    nc.vector.tensor_mul(t2[:], pss[NHC + 1][:], cosR[:, js])
                    nc.vector.tensor_add(roi[:], t1[:], t2[:])
                    for h in range(NHC):
                        hs = slice(h * 32, (h + 1) * 32)
                        nc.scalar.copy(qp[h][0:32, js], ror[hs, :])
                        nc.scalar.copy(qp[h][32:64, js], roi[hs, :])

                    # ---- pass B: kv_a projection (transposed layout) ----
                    kps = [qac.tile([P, SF], F32, tag="acc", name="acc")
                           for _ in range(NCC)]
                    kpr = qac.tile([DR, SF], F32, tag="acc", name="accr",
                                   padded_shape=[P, SF])
                    for d in range(NDC):
                        xs = xh[d // 4][:, (d % 4) * SF:(d % 4 + 1) * SF]
                        for g in range(NCC):
                            nc.tensor.matmul(
                                kps[g][:],
                                wkva_a[:, d * (KVL + DR) + g * P:
                                       d * (KVL + DR) + (g + 1) * P],
                                xs, start=(d == 0), stop=(d == NDC - 1))
                        nc.tensor.matmul(
                            kpr[:],
                            wkva_a[:, d * (KVL + DR) + KVL:
                                   (d + 1) * (KVL + DR)],
                            xs, start=(d == 0), stop=(d == NDC - 1))
                    # rms-norm over latent dim (partition reduction)
                    ssq = qac.tile([1, SF], F32, tag="acc", name="ssq",
                                   padded_shape=[P, SF])
                    for g in range(NCC):
                        sq = sqs.tile([P, SF], BF16, tag="sq", name="sq")
                        nc.scalar.activation(
                            sq[:], kps[g][:],
                            mybir.ActivationFunctionType.Square)
                        nc.tensor.matmul(ssq[:], ones_c[:], sq[:],
                                         start=(g == 0), stop=(g == NCC - 1))
                    rt_ = nrm.tile([1, SF], F32, tag="rt", name="rt")
                    nc.scalar.activation(
                        rt_[:], ssq[:], mybir.ActivationFunctionType.Sqrt,
                        bias=epsb[:], scale=1.0 / KVL)
                    ri = nrm.tile([1, SF], F32, tag="ri", name="ri")
                    nc.vector.reciprocal(ri[:], rt_[:])
                    bcs = nrm.tile([P, SF], F32, tag="bcs", name="bcs")
                    nc.gpsimd.partition_broadcast(bcs[:], ri[:])
                    for g in range(NCC):
                        nc.vector.tensor_mul(kvcT[g][:, js], kps[g][:],
                                             bcs[:])
                    # k rope rotation ([0:32]=even=r, [32:64]=odd=i)
                    m1 = rts.tile([32, SF], F32, tag="m1", name="m1")
                    m2 = rts.tile([32, SF], F32, tag="m2", name="m2")
                    nc.vector.tensor_mul(m1[:], kpr[0:32, :], cosR[0:32, js])
                    nc.vector.tensor_mul(m2[:], kpr[32:64, :],
                                         sinR[0:32, js])
                    nc.vector.tensor_sub(kpeT[0:32, js], m1[:], m2[:])
                    nc.vector.tensor_mul(m1[:], kpr[0:32, :], sinR[0:32, js])
                    nc.vector.tensor_mul(m2[:], kpr[32:64, :],
                                         cosR[0:32, js])
                    nc.vector.tensor_add(kpeT[32:64, js], m1[:], m2[:])

            # ============ phase 2: per-head k_nope/v + attention ===========
            with ExitStack() as ph2:
                wbp = ph2.enter_context(tc.tile_pool(name="wb", bufs=2))
                knp = ph2.enter_context(tc.tile_pool(name="kn", bufs=2))
                vp = ph2.enter_context(tc.tile_pool(name="v", bufs=2))
                etp = ph2.enter_context(tc.tile_pool(name="et", bufs=6))
                dvp = ph2.enter_context(tc.tile_pool(name="dinv", bufs=2))
                msp = ph2.enter_context(
                    tc.tile_pool(name="msp", bufs=3, space="PSUM"))
                ohp = ph2.enter_context(
                    tc.tile_pool(name="oh", bufs=2, space="PSUM"))
                dnp = ph2.enter_context(
                    tc.tile_pool(name="dn", bufs=2, space="PSUM"))

                for h in range(NHC):
                    wbk = wbp.tile([P, NCC * DN], BF16, tag="wbk",
                                   name="wbk")
                    nc.sync.dma_start(
                        wbk[:].rearrange("p (cc dn) -> p cc dn", cc=NCC),
                        d_wbkT.ap()[h].rearrange("(cc p) dn -> p cc dn",
                                                 p=P))
                    wbv = wbp.tile([P, NCC * DV], BF16, tag="wbv",
                                   name="wbv")
                    nc.sync.dma_start(
                        wbv[:].rearrange("p (cc dv) -> p cc dv", cc=NCC),
                        d_wbvT.ap()[h].rearrange("(cc p) dv -> p cc dv",
                                                 p=P))
                    # k_nopeT[dn, t] = sum_cc wbkT_cc^T . kvcT_cc
                    knT = knp.tile([DN, S], BF16, tag="knT", name="knT")
                    for jj in range(NST):
                        jjs = slice(jj * SF, (jj + 1) * SF)
                        ps = msp.tile([P, SF], F32, tag="msp", name="msp")
                        for cc in range(NCC):
                            nc.tensor.matmul(
                                ps[:], wbk[:, cc * DN:(cc + 1) * DN],
                                kvcT[cc][:, jjs],
                                start=(cc == 0), stop=(cc == NCC - 1))
                        nc.scalar.copy(knT[:, jjs], ps[:])
                    # v[t, dv] per 128-t tile, 4 tiles per psum buffer
                    vh = vp.tile([P, NTT * DV], BF16, tag="vh", name="vh")
                    for tq in range(4):
                        ps = msp.tile([P, SF], F32, tag="msp", name="msp")
                        for k in range(4):
                            t = 4 * tq + k
                            ts_ = slice(t * P, (t + 1) * P)
                            for cc in range(NCC):
                                nc.tensor.matmul(
                                    ps[:, k * DV:(k + 1) * DV],
                                    kvcT[cc][:, ts_],
                                    wbv[:, cc * DV:(cc + 1) * DV],
                                    start=(cc == 0), stop=(cc == NCC - 1))
                        nc.scalar.copy(vh[:, tq * SF:(tq + 1) * SF], ps[:])

                    for j in range(NST):
                        js = slice(j * SF, (j + 1) * SF)
                        oh = ohp.tile([DV, SF], F32, tag="oh", name="oh")
                        dn = dnp.tile([1, SF], F32, tag="dn", name="dn",
                                      padded_shape=[P, SF])
                        ntt = 4 * j + 4
                        # software-pipelined: scores(t+1) issued before
                        # dn/oh(t) so the PE never waits on exp(t)
                        scs, es, offs = [None] * ntt, [None] * ntt, [0] * ntt

                        def issue_scores(t):
                            off = max(0, P * (t - 4 * j))
                            nf = SF - off
                            osl = slice(j * SF + off, (j + 1) * SF)
                            sc = msp.tile([P, SF], F32, tag="msp",
                                          name="msp")
                            ts_ = slice(t * P, (t + 1) * P)
                            nc.tensor.matmul(sc[:, 0:nf], knT[:, ts_],
                                             qn[h][:, osl],
                                             start=True, stop=False)
                            nc.tensor.matmul(sc[:, 0:nf], kpeT[:, ts_],
                                             qp[h][:, osl],
                                             start=False, stop=True)
                            e = etp.tile([P, SF], BF16, tag="et", name="et")
                            nc.scalar.activation(
                                e[:, 0:nf], sc[:, 0:nf],
                                mybir.ActivationFunctionType.Exp,
                                scale=SCALE)
                            if t >= 4 * j:
                                nc.gpsimd.affine_select(
                                    out=e[:, 0:nf], in_=e[:, 0:nf],
                                    compare_op=mybir.AluOpType.is_ge,
                                    fill=0.0, base=0, pattern=[[1, nf]],
                                    channel_multiplier=-1)
                            scs[t], es[t], offs[t] = sc, e, off

                        def issue_accum(t):
                            off = offs[t]
                            nf = SF - off
                            e = es[t]
                            nc.tensor.matmul(dn[:, off:SF], ones_c[:],
                                             e[:, 0:nf], start=(t == 0),
                                             stop=(t == ntt - 1))
                            nc.tensor.matmul(
                                oh[:, off:SF],
                                vh[:, t * DV:(t + 1) * DV],
                                e[:, 0:nf], start=(t == 0),
                                stop=(t == ntt - 1))

                        issue_scores(0)
                        for t in range(ntt):
                            if t + 1 < ntt:
                                issue_scores(t + 1)
                            issue_accum(t)

                        di = dvp.tile([1, SF], F32, tag="di", name="di")
                        nc.vector.reciprocal(di[:], dn[:])
                        db = dvp.tile([P, SF], F32, tag="db", name="db")
                        nc.gpsimd.partition_broadcast(db[:], di[:])
                        nc.vector.tensor_mul(oTs[h][:, js], oh[:], db[:])

            # ============ phase 3: output projection (partial) =============
            with ExitStack() as ph3:
                wop = ph3.enter_context(tc.tile_pool(name="wo", bufs=NHC))
                otg = ph3.enter_context(tc.tile_pool(name="ost", bufs=3))
                psp3 = ph3.enter_context(
                    tc.tile_pool(name="ps3", bufs=4, space="PSUM"))
                wos = [wop.tile([DV, DIM], BF16, tag="wo", name="wo")
                       for _ in range(NHC)]
                for h in range(NHC):
                    nc.sync.dma_start(
                        wos[h][:], d_wo.ap()[h * DV:(h + 1) * DV, :])
                for d in range(NDC):
                    ds_ = slice(d * P, (d + 1) * P)
                    obig = otg.tile([P, S], BF16, tag="ost", name="ost")
                    for j in range(NST):
                        js = slice(j * SF, (j + 1) * SF)
                        ps = psp3.tile([P, SF], F32, tag="ps3", name="ps3")
                        for h in range(NHC):
                            nc.tensor.matmul(
                                ps[:], wos[h][:, ds_], oTs[h][:, js],
                                start=(h == 0), stop=(h == NHC - 1))
                        nc.scalar.copy(obig[:, js], ps[:])
                    nc.sync.dma_start(out[ds_, :], obig[:])

    nc.compile()
    return nc


def prep_inputs(x, wq_w, wkv_a_w, wkv_b_w, kv_norm_w, wo_w,
                freqs_cos, freqs_sin):
    """Host-side sharding/layout prep. Returns per-core input maps."""
    bf = NPBF16
    x = np.asarray(x, np.float32).reshape(S, DIM)
    xT = np.ascontiguousarray(x.T).astype(bf)
    wq = np.asarray(wq_w, np.float32).reshape(DIM, NH, DN + DR)
    wkva = np.asarray(wkv_a_w, np.float32)
    # reorder kv_a cols: [latent | rope-even | rope-odd]
    wkva_r = np.concatenate(
        [wkva[:, :KVL], wkva[:, KVL + 0::2], wkva[:, KVL + 1::2]],
        axis=1).astype(bf)
    wkvb = np.asarray(wkv_b_w, np.float32)
    knw = np.asarray(kv_norm_w, np.float32)
    wo = np.asarray(wo_w, np.float32)
    cos = np.asarray(freqs_cos, np.float32)
    sin = np.asarray(freqs_sin, np.float32)
    cosR = np.ascontiguousarray(np.tile(cos.T, (NHC, 1)))  # [128, S]
    sinR = np.ascontiguousarray(np.tile(sin.T, (NHC, 1)))

    maps = []
    for c in range(NCORES):
        hs = list(range(NHC * c, NHC * (c + 1)))
        wq_n = np.ascontiguousarray(
            wq[:, hs, :DN].reshape(DIM, NHC * DN)).astype(bf)
        wq_pr = np.ascontiguousarray(
            wq[:, hs, DN + 0::2].reshape(DIM, NHC * 32)).astype(bf)
        wq_pi = np.ascontiguousarray(
            wq[:, hs, DN + 1::2].reshape(DIM, NHC * 32)).astype(bf)
        # fold kv_norm weight into the absorbed weights (w broadcasts
        # over the latent dim in both the score and value paths)
        wbkT = np.stack(
            [np.ascontiguousarray(
                (wkvb[h * (DN + DV):h * (DN + DV) + DN, :]
                 * knw[None, :]).T) for h in hs])          # [4, 512, 128]
        wbvT = np.stack(
            [np.ascontiguousarray(
                wkvb[h * (DN + DV) + DN:(h + 1) * (DN + DV), :].T)
             * knw[:, None] for h in hs])                  # [4, 512, 128]
        wo_c = np.ascontiguousarray(
            np.concatenate([wo[h * DV:(h + 1) * DV, :] for h in hs]))
        maps.append({
            "xT": xT, "wq_n": wq_n, "wq_pr": wq_pr, "wq_pi": wq_pi,
            "wkv_ar": wkva_r, "wbkT": wbkT.astype(bf),
            "wbvT": wbvT.astype(bf), "wo_c": wo_c.astype(bf),
            "cosR": cosR, "sinR": sinR,
        })
    return maps


def kernel(x, wq_w, wkv_a_w, wkv_b_w, kv_norm_w, wo_w,
           freqs_cos, freqs_sin, start_pos):
    assert int(start_pos) == 0
    maps = prep_inputs(x, wq_w, wkv_a_w, wkv_b_w, kv_norm_w, wo_w,
                       freqs_cos, freqs_sin)
    nc = build_nc()
    res = run_bass_kernel_spmd(nc, maps, list(range(NCORES)))
    acc = np.zeros((DIM, S), np.float64)
    for c in range(NCORES):
        acc += res.results[c]["outT"].astype(np.float64)
    return np.ascontiguousarray(acc.T).astype(np.float32).reshape(1, S, DIM)
